# revision 14
# baseline (speedup 1.0000x reference)
"""Trainium2 Bass kernel for a MoE transformer block (MHLA attention + 8-expert MoE).

Contract: kernel(**inputs) takes the full unsharded fp32 inputs and returns the
full [8, 128, 192] fp32 output. Internally:
  - latent_tokens weight (1.2 GB) is column-sharded across the 8 cores; each
    core computes its slice of the latent, redistributed batch-wise by AllToAll.
  - attention is data-parallel over batch (core c owns batch c).
  - MoE is expert-parallel (core c owns expert c, runs it on all 1024 tokens);
    post-attention activations are AllGathered, gated expert outputs are
    ReduceScattered back to the batch owner.
Matmuls run in bf16 with fp32 PSUM accumulation (router kept in fp32 to keep
top-2 selection faithful); rmsnorm weights are folded into the consuming weight
matrices on the host, biases are folded in as an extra contraction row.
"""

import numpy as np
import ml_dtypes

import concourse.bass as bass
import concourse.bacc as bacc
import concourse.mybir as mybir
from concourse.bass_utils import run_bass_kernel_spmd
from concourse.tile import TileContext
from concourse.masks import make_identity

B, S, E = 8, 128, 192
H, HD = 6, 32
L = 64
NEXP, TOPK, ED = 8, 2, 768
EPS = 1e-5
NC = 8
NL = (L * E) // NC          # latent columns per core = 1536
EB = 4                      # e-tiles per latW DMA batch
f32 = mybir.dt.float32
bf16 = mybir.dt.bfloat16
AF = mybir.ActivationFunctionType
ISCALE = 1.0 / np.sqrt(HD).item()

_cache = {}


def _build_program():
    nc = bacc.Bacc("TRN2", target_bir_lowering=False, debug=False, num_devices=NC)

    X = nc.declare_dram_parameter("X", [B, S, E], f32, isOutput=False)
    XB = nc.declare_dram_parameter("XB", [S, E], f32, isOutput=False)
    LATW = nc.declare_dram_parameter("LATW", [E, S, NL], bf16, isOutput=False)
    LATB = nc.declare_dram_parameter("LATB", [1, NL], bf16, isOutput=False)
    QW = nc.declare_dram_parameter("QW", [E + 1, E], bf16, isOutput=False)
    KW = nc.declare_dram_parameter("KW", [E + 1, E], bf16, isOutput=False)
    VW = nc.declare_dram_parameter("VW", [E + 1, E], bf16, isOutput=False)
    OW = nc.declare_dram_parameter("OW", [E + 1, E], bf16, isOutput=False)
    COS = nc.declare_dram_parameter("COS", [S, E // 2], f32, isOutput=False)
    SIN = nc.declare_dram_parameter("SIN", [S, E // 2], f32, isOutput=False)
    RW = nc.declare_dram_parameter("RW", [E + 1, NEXP], f32, isOutput=False)
    E1W = nc.declare_dram_parameter("E1W", [E + 1, ED], bf16, isOutput=False)
    SWW = nc.declare_dram_parameter("SWW", [ED + 1, ED], bf16, isOutput=False)
    E2W = nc.declare_dram_parameter("E2W", [ED + 1, E], bf16, isOutput=False)
    YB = nc.declare_dram_parameter("YB", [S, E], f32, isOutput=True)

    groups = [list(range(NC))]

    from contextlib import ExitStack
    with TileContext(nc) as tc, ExitStack() as ctx:
        const = ctx.enter_context(tc.tile_pool(name="const", bufs=1))
        dram = ctx.enter_context(tc.tile_pool(name="dram", bufs=1, space="DRAM"))
        wpool = ctx.enter_context(tc.tile_pool(name="w", bufs=1))
        lat_stream = ctx.enter_context(tc.tile_pool(name="lats", bufs=4))
        sb = ctx.enter_context(tc.tile_pool(name="sb", bufs=1))
        work = ctx.enter_context(tc.tile_pool(name="work", bufs=2))
        psA = ctx.enter_context(tc.tile_pool(name="psA", bufs=1, space="PSUM"))
        psP = ctx.enter_context(tc.tile_pool(name="psP", bufs=1, space="PSUM"))

        # ---- constants ----
        idf = const.tile([128, 128], f32, tag="idf")
        make_identity(nc, idf[:])
        idb = const.tile([128, 128], bf16, tag="idb")
        make_identity(nc, idb[:])
        ones_b = const.tile([1, B], bf16, tag="ones_b")
        nc.vector.memset(ones_b[:], 1.0)
        ones_s = const.tile([1, 2 * S], bf16, tag="ones_s")
        nc.vector.memset(ones_s[:], 1.0)
        eps_t = const.tile([128, 1], f32, tag="eps_t")
        nc.vector.memset(eps_t[:], EPS)

        # ---- dram scratch (collective outputs in Shared space) ----
        a2a_in = dram.tile([B, NL], f32, tag="a2a_in")
        a2a_out = dram.tile([B, NL], f32, tag="a2a_out")
        ag_in = dram.tile([S, E], f32, tag="ag_in")
        h_all = dram.tile([B * S, E], f32, tag="h_all", addr_space="Shared")
        rs_in = dram.tile([B * S, E], f32, tag="rs_in")
        rs_out = dram.tile([S, E], f32, tag="rs_out")

        scopeA = nc.named_scope("phaseA"); scopeA.__enter__()
        # =========== Phase A: latent = rmsnorm(x) @ latW (column shard) ===========
        # x in [s, (b, e)] layout
        x_all = sb.tile([S, B * E], f32, tag="x_all")
        nc.sync.dma_start(
            out=x_all[:].rearrange("s (b e) -> s b e", b=B),
            in_=X[:].rearrange("b s e -> s b e"),
        )
        msq = sb.tile([S, B], f32, tag="msq")
        for b in range(B):
            sq_scr = work.tile([S, E], f32, tag="sq_scr")
            nc.scalar.activation(
                sq_scr[:], x_all[:, b * E:(b + 1) * E], AF.Square,
                accum_out=msq[:, b:b + 1],
            )
        rroot = sb.tile([S, B], f32, tag="rroot")
        nc.scalar.activation(rroot[:], msq[:], AF.Sqrt, scale=1.0 / E, bias=eps_t[:])
        rinv = sb.tile([S, B], f32, tag="rinv")
        nc.vector.reciprocal(rinv[:], rroot[:])
        xn_bf = sb.tile([S, B * E], bf16, tag="xn_bf")
        for b in range(B):
            nc.scalar.activation(
                xn_bf[:, b * E:(b + 1) * E], x_all[:, b * E:(b + 1) * E],
                AF.Copy, scale=rinv[:, b:b + 1],
            )
        xn_r = xn_bf[:].rearrange("s (b e) -> s e b", b=B)  # [S, E, B]

        latb_sb = const.tile([1, NL], bf16, tag="latb_sb")
        nc.sync.dma_start(out=latb_sb[:], in_=LATB[:])

        NT = [(0, 512), (512, 512), (1024, 512)]
        lat_ps = [psA.tile([B, 512], f32, tag=f"lat_ps{j}", name=f"lat_ps{j}")
                  for j in range(3)]
        for eb in range(E // EB):
            wt = lat_stream.tile([S, EB * NL], bf16, tag="wt")
            eng = nc.sync if (eb % 2 == 0) else nc.scalar
            eng.dma_start(
                out=wt[:].rearrange("s (f n) -> s f n", f=EB),
                in_=LATW[eb * EB:(eb + 1) * EB].rearrange("f s n -> s f n"),
            )
            for f in range(EB):
                e = eb * EB + f
                lhsT = xn_r[:, e, :]  # [S, B] strided
                for j, (n0, nn) in enumerate(NT):
                    nc.tensor.matmul(
                        lat_ps[j][:], lhsT, wt[:, f * NL + n0:f * NL + n0 + nn],
                        start=(e == 0), stop=False,
                    )
        for j, (n0, nn) in enumerate(NT):
            nc.tensor.matmul(
                lat_ps[j][:], ones_b[:], latb_sb[:, n0:n0 + nn],
                start=False, stop=True,
            )
        lat_part = sb.tile([B, NL], f32, tag="lat_part")
        for j, (n0, nn) in enumerate(NT):
            nc.scalar.copy(lat_part[:, n0:n0 + nn], lat_ps[j][:])
        nc.sync.dma_start(out=a2a_in[:], in_=lat_part[:])
        nc.gpsimd.collective_compute(
            "AllToAll", mybir.AluOpType.bypass, replica_groups=groups,
            ins=[a2a_in.opt()], outs=[a2a_out.opt()],
        )

        scopeA.__exit__(None, None, None)
        scopeB = nc.named_scope("phaseB"); scopeB.__enter__()
        # =========== Phase B: attention for this core's batch ===========
        def load_w2(name, W, n, dt):
            w0 = wpool.tile([96, n], dt, tag=name + "0", name=name + "0")
            nc.scalar.dma_start(out=w0[:], in_=W[0:96, :])
            w1 = wpool.tile([97, n], dt, tag=name + "1", name=name + "1")
            nc.scalar.dma_start(out=w1[:], in_=W[96:193, :])
            return w0, w1

        qw0, qw1 = load_w2("qw", QW, E, bf16)
        kw0, kw1 = load_w2("kw", KW, E, bf16)
        vw0, vw1 = load_w2("vw", VW, E, bf16)
        ow0, ow1 = load_w2("ow", OW, E, bf16)
        cos_sb = wpool.tile([S, E // 2], f32, tag="cos_sb")
        nc.scalar.dma_start(out=cos_sb[:], in_=COS[:])
        sin_sb = wpool.tile([S, E // 2], f32, tag="sin_sb")
        nc.scalar.dma_start(out=sin_sb[:], in_=SIN[:])

        xb = sb.tile([S, E], f32, tag="xb")
        nc.scalar.dma_start(out=xb[:], in_=XB[:])
        # rmsnorm(xb)
        sqb = work.tile([S, E], f32, tag="sqb")
        msb = sb.tile([S, 1], f32, tag="msb")
        nc.scalar.activation(sqb[:], xb[:], AF.Square, accum_out=msb[:])
        rrb = sb.tile([S, 1], f32, tag="rrb")
        nc.scalar.activation(rrb[:], msb[:], AF.Sqrt, scale=1.0 / E, bias=eps_t[:])
        rib = sb.tile([S, 1], f32, tag="rib")
        nc.vector.reciprocal(rib[:], rrb[:])
        xnb = sb.tile([S, E], f32, tag="xnb")
        nc.scalar.activation(xnb[:], xb[:], AF.Copy, scale=rib[:])

        # transpose xnb -> [e, s] bf16 K-tiles (96 rows + 97 rows with ones)
        xnbT0 = sb.tile([96, S], bf16, tag="xnbT0")
        xnbT1 = sb.tile([97, S], bf16, tag="xnbT1")
        pt0 = psP.tile([96, S], f32, tag="tp", bufs=2)
        nc.tensor.transpose(pt0[:], xnb[:, 0:96], idf[:])
        nc.vector.tensor_copy(xnbT0[:], pt0[:])
        pt1 = psP.tile([96, S], f32, tag="tp", bufs=2)
        nc.tensor.transpose(pt1[:], xnb[:, 96:192], idf[:])
        nc.vector.tensor_copy(xnbT1[0:96, :], pt1[:])
        nc.vector.memset(xnbT1[96:97, :], 1.0)

        # q = xnb @ qW + qb  -> [S, E]
        psq = psP.tile([S, E], f32, tag="mm", bufs=3)
        nc.tensor.matmul(psq[:], xnbT0[:], qw0[:], start=True, stop=False)
        nc.tensor.matmul(psq[:], xnbT1[:], qw1[:], start=False, stop=True)
        # RoPE directly from psum -> bf16 qr
        qr_bf = sb.tile([S, E], bf16, tag="qr_bf")
        q_pair = psq[:].rearrange("s (p two) -> s two p", two=2)
        qr_pair = qr_bf[:].rearrange("s (p two) -> s two p", two=2)
        t_a = work.tile([S, E // 2], f32, tag="t_a")
        t_b = work.tile([S, E // 2], f32, tag="t_b")
        nc.vector.tensor_mul(t_a[:], q_pair[:, 0, :], cos_sb[:])
        nc.vector.tensor_mul(t_b[:], q_pair[:, 1, :], sin_sb[:])
        nc.vector.tensor_sub(qr_pair[:, 0, :], t_a[:], t_b[:])
        t_c = work.tile([S, E // 2], f32, tag="t_c")
        t_d = work.tile([S, E // 2], f32, tag="t_d")
        nc.vector.tensor_mul(t_c[:], q_pair[:, 0, :], sin_sb[:])
        nc.vector.tensor_mul(t_d[:], q_pair[:, 1, :], cos_sb[:])
        nc.vector.tensor_add(qr_pair[:, 1, :], t_c[:], t_d[:])
        # transpose qr -> qrT [e, s] bf16
        qrT0 = sb.tile([96, S], bf16, tag="qrT0")
        qrT1 = sb.tile([96, S], bf16, tag="qrT1")
        ptq0 = psP.tile([96, S], bf16, tag="tp", bufs=2)
        nc.tensor.transpose(ptq0[:], qr_bf[:, 0:96], idb[:])
        nc.vector.tensor_copy(qrT0[:], ptq0[:])
        ptq1 = psP.tile([96, S], bf16, tag="tp", bufs=2)
        nc.tensor.transpose(ptq1[:], qr_bf[:, 96:192], idb[:])
        nc.vector.tensor_copy(qrT1[:], ptq1[:])

        # latent for this batch: [L, E] contiguous in a2a_out
        lat = sb.tile([L, E], f32, tag="lat")
        nc.sync.dma_start(out=lat[:], in_=a2a_out[:].rearrange("b n -> (b n)").rearrange("(l e) -> l e", e=E))
        latT0 = sb.tile([96, L], bf16, tag="latT0")
        latT1 = sb.tile([97, L], bf16, tag="latT1")
        ptl0 = psP.tile([96, L], f32, tag="tp", bufs=2)
        nc.tensor.transpose(ptl0[:], lat[:, 0:96], idf[0:64, 0:64])
        nc.vector.tensor_copy(latT0[:], ptl0[:])
        ptl1 = psP.tile([96, L], f32, tag="tp", bufs=2)
        nc.tensor.transpose(ptl1[:], lat[:, 96:192], idf[0:64, 0:64])
        nc.vector.tensor_copy(latT1[0:96, :], ptl1[:])
        nc.vector.memset(latT1[96:97, :], 1.0)

        # kT [e_out, l] bf16 (2 M-slices); v [l, e_out] bf16
        kT0 = sb.tile([96, L], bf16, tag="kT0")
        kT1 = sb.tile([96, L], bf16, tag="kT1")
        for mi, kT in enumerate((kT0, kT1)):
            pk = psP.tile([96, L], f32, tag="mm", bufs=3)
            nc.tensor.matmul(pk[:], kw0[:, mi * 96:(mi + 1) * 96], latT0[:],
                             start=True, stop=False)
            nc.tensor.matmul(pk[:], kw1[:, mi * 96:(mi + 1) * 96], latT1[:],
                             start=False, stop=True)
            nc.vector.tensor_copy(kT[:], pk[:])
        v_sb = sb.tile([L, E], bf16, tag="v_sb")
        pv = psP.tile([L, E], f32, tag="mm", bufs=3)
        nc.tensor.matmul(pv[:], latT0[:], vw0[:], start=True, stop=False)
        nc.tensor.matmul(pv[:], latT1[:], vw1[:], start=False, stop=True)
        nc.vector.tensor_copy(v_sb[:], pv[:])

        # attention heads
        aoT0 = sb.tile([96, S], bf16, tag="aoT0")
        aoT1 = sb.tile([97, S], bf16, tag="aoT1")
        nc.vector.memset(aoT1[96:97, :], 1.0)
        qrTs, kTs, aoTs = (qrT0, qrT1), (kT0, kT1), (aoT0, aoT1)
        for h in range(H):
            ht, hr = divmod(h, 3)
            pss = psP.tile([S, L], f32, tag="mm", bufs=3)
            nc.tensor.matmul(pss[:], qrTs[ht][hr * 32:(hr + 1) * 32, :],
                             kTs[ht][hr * 32:(hr + 1) * 32, :], start=True, stop=True)
            mx = work.tile([S, 1], f32, tag="mx")
            nc.vector.reduce_max(out=mx[:], in_=pss[:], axis=mybir.AxisListType.X)
            nbias = work.tile([S, 1], f32, tag="nbias")
            nc.vector.tensor_scalar_mul(nbias[:], mx[:], -ISCALE)
            p_sb = work.tile([S, L], f32, tag="p_sb")
            ssum = work.tile([S, 1], f32, tag="ssum")
            nc.scalar.activation(p_sb[:], pss[:], AF.Exp, scale=ISCALE,
                                 bias=nbias[:], accum_out=ssum[:])
            rsum = work.tile([S, 1], f32, tag="rsum")
            nc.vector.reciprocal(rsum[:], ssum[:])
            at_bf = work.tile([S, L], bf16, tag="at_bf")
            nc.vector.tensor_scalar_mul(at_bf[:], p_sb[:], rsum[:])
            psat = psP.tile([L, S], bf16, tag="tp", bufs=2)
            nc.tensor.transpose(psat[:], at_bf[:], idb[:])
            atT = work.tile([L, S], bf16, tag="atT")
            nc.vector.tensor_copy(atT[:], psat[:])
            pso = psP.tile([32, S], f32, tag="mm", bufs=3)
            nc.tensor.matmul(pso[:], v_sb[:, h * 32:(h + 1) * 32], atT[:],
                             start=True, stop=True)
            nc.vector.tensor_copy(aoTs[ht][hr * 32:(hr + 1) * 32, :], pso[:])

        # o proj + residual
        pso1 = psP.tile([S, E], f32, tag="mm", bufs=3)
        nc.tensor.matmul(pso1[:], aoT0[:], ow0[:], start=True, stop=False)
        nc.tensor.matmul(pso1[:], aoT1[:], ow1[:], start=False, stop=True)
        out1 = sb.tile([S, E], f32, tag="out1")
        nc.vector.tensor_add(out1[:], pso1[:], xb[:])
        nc.sync.dma_start(out=ag_in[:], in_=out1[:])
        nc.gpsimd.collective_compute(
            "AllGather", mybir.AluOpType.bypass, replica_groups=groups,
            ins=[ag_in.opt()], outs=[h_all.opt()],
        )

        scopeB.__exit__(None, None, None)
        scopeC = nc.named_scope("phaseC"); scopeC.__enter__()
        # =========== Phase C: MoE expert for this core, all tokens ===========
        rw0, rw1 = load_w2("rw", RW, NEXP, f32)
        e1w0, e1w1 = load_w2("e1w", E1W, ED, bf16)
        sww_k = []
        for k in range(6):
            t = wpool.tile([128, ED], bf16, tag=f"sww{k}", name=f"sww{k}")
            nc.scalar.dma_start(out=t[:], in_=SWW[k * 128:(k + 1) * 128, :])
            sww_k.append(t)
        swb_sb = wpool.tile([1, ED], bf16, tag="swb_sb")
        nc.scalar.dma_start(out=swb_sb[:], in_=SWW[ED:ED + 1, :])
        e2w_k = []
        for k in range(6):
            t = wpool.tile([128, E], bf16, tag=f"e2w{k}", name=f"e2w{k}")
            nc.scalar.dma_start(out=t[:], in_=E2W[k * 128:(k + 1) * 128, :])
            e2w_k.append(t)
        e2b_sb = wpool.tile([1, E], bf16, tag="e2b_sb")
        nc.scalar.dma_start(out=e2b_sb[:], in_=E2W[ED:ED + 1, :])

        S2 = 2 * S  # token-tile pair width
        for tp in range(B // 2):
            hin = work.tile([S, 2 * E], f32, tag="hin")  # two tiles side by side
            nc.sync.dma_start(
                out=hin[:].rearrange("s (u e) -> s u e", u=2),
                in_=h_all[2 * tp * S:(2 * tp + 2) * S, :].rearrange("(u s) e -> s u e", u=2),
            )
            msm = work.tile([S, 2], f32, tag="msm")
            for u in range(2):
                sqm = work.tile([S, E], f32, tag="sqm")
                nc.scalar.activation(sqm[:], hin[:, u * E:(u + 1) * E], AF.Square,
                                     accum_out=msm[:, u:u + 1])
            rrm = work.tile([S, 2], f32, tag="rrm")
            nc.scalar.activation(rrm[:], msm[:], AF.Sqrt, scale=1.0 / E, bias=eps_t[:])
            rim = work.tile([S, 2], f32, tag="rim")
            nc.vector.reciprocal(rim[:], rrm[:])
            hn = work.tile([S, 2 * E], f32, tag="hn")
            for u in range(2):
                nc.scalar.activation(hn[:, u * E:(u + 1) * E], hin[:, u * E:(u + 1) * E],
                                     AF.Copy, scale=rim[:, u:u + 1])

            # transposes: [e, (u s)] bf16 + f32; rows 0:96 and 96:192(+ones)
            hnT0 = work.tile([96, S2], bf16, tag="hnT0")
            hnT1 = work.tile([97, S2], bf16, tag="hnT1")
            hnTf0 = work.tile([96, S2], f32, tag="hnTf0")
            hnTf1 = work.tile([97, S2], f32, tag="hnTf1")
            for u in range(2):
                ptm0 = psP.tile([96, S], f32, tag="tp", bufs=2)
                nc.tensor.transpose(ptm0[:], hn[:, u * E:u * E + 96], idf[:])
                nc.vector.tensor_copy(hnT0[:, u * S:(u + 1) * S], ptm0[:])
                nc.scalar.copy(hnTf0[:, u * S:(u + 1) * S], ptm0[:])
                ptm1 = psP.tile([96, S], f32, tag="tp", bufs=2)
                nc.tensor.transpose(ptm1[:], hn[:, u * E + 96:u * E + 192], idf[:])
                nc.vector.tensor_copy(hnT1[0:96, u * S:(u + 1) * S], ptm1[:])
                nc.scalar.copy(hnTf1[0:96, u * S:(u + 1) * S], ptm1[:])
            nc.vector.memset(hnT1[96:97, :], 1.0)
            nc.vector.memset(hnTf1[96:97, :], 1.0)

            # router (fp32): logits -> softmax -> top2 weight for expert 0 (rotated)
            plog = psP.tile([S, 2 * NEXP], f32, tag="mm", bufs=3)
            for u in range(2):
                nc.tensor.matmul(plog[:, u * NEXP:(u + 1) * NEXP],
                                 hnTf0[:, u * S:(u + 1) * S], rw0[:],
                                 start=True, stop=False)
                nc.tensor.matmul(plog[:, u * NEXP:(u + 1) * NEXP],
                                 hnTf1[:, u * S:(u + 1) * S], rw1[:],
                                 start=False, stop=True)
            w0 = work.tile([S, 2], f32, tag="w0")
            for u in range(2):
                pl = plog[:, u * NEXP:(u + 1) * NEXP]
                mx1 = work.tile([S, 1], f32, tag="mx1")
                nc.vector.reduce_max(out=mx1[:], in_=pl, axis=mybir.AxisListType.X)
                nb1 = work.tile([S, 1], f32, tag="nb1")
                nc.vector.tensor_scalar_mul(nb1[:], mx1[:], -1.0)
                pr = work.tile([S, NEXP], f32, tag="pr")
                sden = work.tile([S, 1], f32, tag="sden")
                nc.scalar.activation(pr[:], pl, AF.Exp, bias=nb1[:], accum_out=sden[:])
                rden = work.tile([S, 1], f32, tag="rden")
                nc.vector.reciprocal(rden[:], sden[:])
                probs = work.tile([S, NEXP], f32, tag="probs")
                nc.vector.tensor_scalar_mul(probs[:], pr[:], rden[:])
                pmax = work.tile([S, 1], f32, tag="pmax")
                nc.vector.reduce_max(out=pmax[:], in_=probs[:], axis=mybir.AxisListType.X)
                eqm = work.tile([S, NEXP], f32, tag="eqm")
                nc.vector.tensor_scalar(eqm[:], probs[:], pmax[:], None,
                                        op0=mybir.AluOpType.is_ge)
                knock = work.tile([S, NEXP], f32, tag="knock")
                nc.vector.tensor_scalar_mul(knock[:], eqm[:], 2.0)
                probs2 = work.tile([S, NEXP], f32, tag="probs2")
                nc.vector.tensor_sub(probs2[:], probs[:], knock[:])
                mx2 = work.tile([S, 1], f32, tag="mx2")
                nc.vector.reduce_max(out=mx2[:], in_=probs2[:], axis=mybir.AxisListType.X)
                m0 = work.tile([S, 1], f32, tag="m0")
                nc.vector.tensor_tensor(out=m0[:], in0=probs[:, 0:1], in1=mx2[:],
                                        op=mybir.AluOpType.is_ge)
                nc.vector.tensor_mul(w0[:, u:u + 1], probs[:, 0:1], m0[:])

            # expert: t1T slices (bf16), then t2T with relu, then eo token-major
            t1T = []
            for m in range(6):
                ps1 = psP.tile([128, S2], f32, tag="mm", bufs=3)
                nc.tensor.matmul(ps1[:], e1w0[:, m * 128:(m + 1) * 128], hnT0[:],
                                 start=True, stop=False)
                nc.tensor.matmul(ps1[:], e1w1[:, m * 128:(m + 1) * 128], hnT1[:],
                                 start=False, stop=True)
                t = work.tile([128, S2], bf16, tag=f"t1T{m}", name=f"t1T{m}")
                nc.vector.tensor_copy(t[:], ps1[:])
                t1T.append(t)
            t2T = []
            for m in range(6):
                ps2 = psP.tile([128, S2], f32, tag="mm", bufs=3)
                for k in range(6):
                    nc.tensor.matmul(ps2[:], sww_k[k][:, m * 128:(m + 1) * 128],
                                     t1T[k][:], start=(k == 0), stop=False)
                nc.tensor.matmul(ps2[:], swb_sb[:, m * 128:(m + 1) * 128], ones_s[:],
                                 start=False, stop=True)
                t = work.tile([128, S2], bf16, tag=f"t2T{m}", name=f"t2T{m}")
                nc.scalar.activation(t[:], ps2[:], AF.Relu)
                t2T.append(t)
            for u in range(2):
                pse = psP.tile([S, E], f32, tag="mm", bufs=3)
                for m in range(6):
                    nc.tensor.matmul(pse[:], t2T[m][:, u * S:(u + 1) * S], e2w_k[m][:],
                                     start=(m == 0), stop=False)
                nc.tensor.matmul(pse[:], ones_s[:, 0:S], e2b_sb[:],
                                 start=False, stop=True)
                y_t = work.tile([S, E], f32, tag="y_t")
                nc.vector.tensor_scalar_mul(y_t[:], pse[:], w0[:, u:u + 1])
                nc.sync.dma_start(out=rs_in[(2 * tp + u) * S:(2 * tp + u + 1) * S, :],
                                  in_=y_t[:])

        nc.gpsimd.collective_compute(
            "ReduceScatter", mybir.AluOpType.add, replica_groups=groups,
            ins=[rs_in.opt()], outs=[rs_out.opt()],
        )
        rsb = sb.tile([S, E], f32, tag="rsb")
        nc.sync.dma_start(out=rsb[:], in_=rs_out[:])
        fin = sb.tile([S, E], f32, tag="fin")
        nc.vector.tensor_add(fin[:], out1[:], rsb[:])
        nc.sync.dma_start(out=YB[:], in_=fin[:])
        scopeC.__exit__(None, None, None)

    nc.compile()
    return nc


def _prep_inputs(inputs):
    bf = ml_dtypes.bfloat16
    x = np.ascontiguousarray(inputs["x"], dtype=np.float32)
    rms1 = np.asarray(inputs["rms1_w"], dtype=np.float32)
    rms2 = np.asarray(inputs["rms2_w"], dtype=np.float32)
    latW = np.asarray(inputs["latW"], dtype=np.float32)
    latb = np.asarray(inputs["latb"], dtype=np.float32)
    qW, qb = np.asarray(inputs["qW"], np.float32), np.asarray(inputs["qb"], np.float32)
    kW, kb = np.asarray(inputs["kW"], np.float32), np.asarray(inputs["kb"], np.float32)
    vW, vb = np.asarray(inputs["vW"], np.float32), np.asarray(inputs["vb"], np.float32)
    oW, ob = np.asarray(inputs["oW"], np.float32), np.asarray(inputs["ob"], np.float32)
    rW, rb = np.asarray(inputs["rW"], np.float32), np.asarray(inputs["rb"], np.float32)
    e1W, e1b = np.asarray(inputs["e1W"], np.float32), np.asarray(inputs["e1b"], np.float32)
    swW, swb = np.asarray(inputs["swW"], np.float32), np.asarray(inputs["swb"], np.float32)
    e2W, e2b = np.asarray(inputs["e2W"], np.float32), np.asarray(inputs["e2b"], np.float32)

    # latW rows are (s, e)-major; fold rms1[e], permute to (e, s)-major, shard cols
    latWp = (latW.reshape(S, E, L * E) * rms1[None, :, None]).transpose(1, 0, 2)
    latW_bf = latWp.astype(bf)  # [E, S, L*E]

    qWp = np.concatenate([qW * rms1[:, None], qb[None]], 0).astype(bf)
    kWp = np.concatenate([kW, kb[None]], 0).astype(bf)
    vWp = np.concatenate([vW, vb[None]], 0).astype(bf)
    oWp = np.concatenate([oW, ob[None]], 0).astype(bf)
    rW_eff = np.concatenate([rW * rms2[:, None], rb[None]], 0).astype(np.float32)

    t = np.arange(S, dtype=np.float64)
    inv_freq = 1.0 / (10000.0 ** (np.arange(0, HD, 2, dtype=np.float64) / HD))
    freqs = np.outer(t, inv_freq)  # [S, 16]
    cos_full = np.tile(np.cos(freqs), (1, H)).astype(np.float32)
    sin_full = np.tile(np.sin(freqs), (1, H)).astype(np.float32)

    in_maps = []
    for c in range(NC):
        cols = np.roll(np.arange(NEXP), -c)
        m = {
            "X": x,
            "XB": np.ascontiguousarray(x[c]),
            "LATW": np.ascontiguousarray(latW_bf[:, :, c * NL:(c + 1) * NL]),
            "LATB": latb[c * NL:(c + 1) * NL].astype(bf)[None, :],
            "QW": qWp, "KW": kWp, "VW": vWp, "OW": oWp,
            "COS": cos_full, "SIN": sin_full,
            "RW": np.ascontiguousarray(rW_eff[:, cols]),
            "E1W": np.concatenate([e1W[c] * rms2[:, None], e1b[c][None]], 0).astype(bf),
            "SWW": np.concatenate([swW[c], swb[c][None]], 0).astype(bf),
            "E2W": np.concatenate([e2W[c], e2b[c][None]], 0).astype(bf),
        }
        in_maps.append(m)
    return in_maps


def _get_program():
    if "nc" not in _cache:
        _cache["nc"] = _build_program()
    return _cache["nc"]


def run(inputs, trace=False):
    nc = _get_program()
    in_maps = _prep_inputs(inputs)
    res = run_bass_kernel_spmd(nc, in_maps, list(range(NC)), trace=trace)
    out = np.stack([res.results[c]["YB"] for c in range(NC)], axis=0)
    return out.astype(np.float32), res


def kernel(**inputs):
    out, _ = run(inputs, trace=False)
    return out


# revision 17
# speedup vs baseline: 1.0508x; 1.0508x over previous
"""Trainium2 Bass kernel for a MoE transformer block (MHLA attention + 8-expert MoE).

Contract: kernel(**inputs) takes the full unsharded fp32 inputs and returns the
full [8, 128, 192] fp32 output. Internally:
  - latent_tokens weight (1.2 GB) is column-sharded across the 8 cores; each
    core computes its slice of the latent, redistributed batch-wise by AllToAll.
  - attention is data-parallel over batch (core c owns batch c).
  - MoE is expert-parallel (core c owns expert c, runs it on all 1024 tokens);
    post-attention activations are AllGathered, gated expert outputs are
    ReduceScattered back to the batch owner.
Matmuls run in bf16 with fp32 PSUM accumulation (router kept in fp32 to keep
top-2 selection faithful); rmsnorm weights are folded into the consuming weight
matrices on the host, biases are folded in as an extra contraction row.
"""

import numpy as np
import ml_dtypes

import concourse.bass as bass
import concourse.bacc as bacc
import concourse.mybir as mybir
from concourse.bass_utils import run_bass_kernel_spmd
from concourse.tile import TileContext
from concourse.masks import make_identity

B, S, E = 8, 128, 192
H, HD = 6, 32
L = 64
NEXP, TOPK, ED = 8, 2, 768
EPS = 1e-5
NC = 8
NL = (L * E) // NC          # latent columns per core = 1536
EB = 4                      # e-tiles per latW DMA batch
f32 = mybir.dt.float32
bf16 = mybir.dt.bfloat16
AF = mybir.ActivationFunctionType
ISCALE = 1.0 / np.sqrt(HD).item()

_cache = {}


def _build_program():
    nc = bacc.Bacc("TRN2", target_bir_lowering=False, debug=False, num_devices=NC)

    X = nc.declare_dram_parameter("X", [B, S, E], f32, isOutput=False)
    XB = nc.declare_dram_parameter("XB", [S, E], f32, isOutput=False)
    LATW = nc.declare_dram_parameter("LATW", [E, S, NL], bf16, isOutput=False)
    LATB = nc.declare_dram_parameter("LATB", [1, NL], bf16, isOutput=False)
    QW = nc.declare_dram_parameter("QW", [E + 1, E], bf16, isOutput=False)
    KW = nc.declare_dram_parameter("KW", [E + 1, E], bf16, isOutput=False)
    VW = nc.declare_dram_parameter("VW", [E + 1, E], bf16, isOutput=False)
    OW = nc.declare_dram_parameter("OW", [E + 1, E], bf16, isOutput=False)
    COS = nc.declare_dram_parameter("COS", [S, E // 2], f32, isOutput=False)
    SIN = nc.declare_dram_parameter("SIN", [S, E // 2], f32, isOutput=False)
    RW = nc.declare_dram_parameter("RW", [E + 1, NEXP], f32, isOutput=False)
    E1W = nc.declare_dram_parameter("E1W", [E + 1, ED], bf16, isOutput=False)
    SWW = nc.declare_dram_parameter("SWW", [ED + 1, ED], bf16, isOutput=False)
    E2W = nc.declare_dram_parameter("E2W", [ED + 1, E], bf16, isOutput=False)
    YB = nc.declare_dram_parameter("YB", [S, E], f32, isOutput=True)

    groups = [list(range(NC))]

    from contextlib import ExitStack
    with TileContext(nc) as tc, ExitStack() as ctx:
        const = ctx.enter_context(tc.tile_pool(name="const", bufs=1))
        dram = ctx.enter_context(tc.tile_pool(name="dram", bufs=1, space="DRAM"))
        wpool = ctx.enter_context(tc.tile_pool(name="w", bufs=1))
        lat_stream = ctx.enter_context(tc.tile_pool(name="lats", bufs=6))
        sb = ctx.enter_context(tc.tile_pool(name="sb", bufs=1))
        work = ctx.enter_context(tc.tile_pool(name="work", bufs=2))
        psA = ctx.enter_context(tc.tile_pool(name="psA", bufs=1, space="PSUM"))
        psP = ctx.enter_context(tc.tile_pool(name="psP", bufs=1, space="PSUM"))

        # ---- constants ----
        idf = const.tile([128, 128], f32, tag="idf")
        make_identity(nc, idf[:])
        idb = const.tile([128, 128], bf16, tag="idb")
        make_identity(nc, idb[:])
        ones_b = const.tile([1, B], bf16, tag="ones_b")
        nc.vector.memset(ones_b[:], 1.0)
        ones_s = const.tile([1, 2 * S], bf16, tag="ones_s")
        nc.vector.memset(ones_s[:], 1.0)
        eps_t = const.tile([128, 1], f32, tag="eps_t")
        nc.vector.memset(eps_t[:], EPS)

        # ---- dram scratch (collective outputs in Shared space) ----
        a2a_in = dram.tile([B, NL], f32, tag="a2a_in")
        a2a_out = dram.tile([B, NL], f32, tag="a2a_out")
        ag_in = dram.tile([S, E], f32, tag="ag_in")
        h_all = dram.tile([B * S, E], f32, tag="h_all", addr_space="Shared")
        rs_in = dram.tile([B * S, E], f32, tag="rs_in")
        rs_out = dram.tile([S, E], f32, tag="rs_out")

        scopeA = nc.named_scope("phaseA"); scopeA.__enter__()
        # =========== Phase A: latent = rmsnorm(x) @ latW (column shard) ===========
        # x in [s, (b, e)] layout
        x_all = sb.tile([S, B * E], f32, tag="x_all")
        nc.sync.dma_start(
            out=x_all[:].rearrange("s (b e) -> s b e", b=B),
            in_=X[:].rearrange("b s e -> s b e"),
        )
        msq = sb.tile([S, B], f32, tag="msq")
        for b in range(B):
            sq_scr = work.tile([S, E], f32, tag="sq_scr")
            nc.scalar.activation(
                sq_scr[:], x_all[:, b * E:(b + 1) * E], AF.Square,
                accum_out=msq[:, b:b + 1],
            )
        rroot = sb.tile([S, B], f32, tag="rroot")
        nc.scalar.activation(rroot[:], msq[:], AF.Sqrt, scale=1.0 / E, bias=eps_t[:])
        rinv = sb.tile([S, B], f32, tag="rinv")
        nc.vector.reciprocal(rinv[:], rroot[:])
        xn_bf = sb.tile([S, B * E], bf16, tag="xn_bf")
        for b in range(B):
            nc.vector.tensor_scalar_mul(
                xn_bf[:, b * E:(b + 1) * E], x_all[:, b * E:(b + 1) * E],
                rinv[:, b:b + 1],
            )
        xn_r = xn_bf[:].rearrange("s (b e) -> s e b", b=B)  # [S, E, B]

        latb_sb = const.tile([1, NL], bf16, tag="latb_sb")
        nc.sync.dma_start(out=latb_sb[:], in_=LATB[:])

        NT = [(0, 512), (512, 512), (1024, 512)]
        lat_ps = [psA.tile([B, 512], f32, tag=f"lat_ps{j}", name=f"lat_ps{j}")
                  for j in range(3)]
        for eb in range(E // EB):
            wt = lat_stream.tile([S, EB * NL], bf16, tag="wt")
            eng = nc.sync if (eb % 2 == 0) else nc.scalar
            eng.dma_start(
                out=wt[:].rearrange("s (f n) -> s f n", f=EB),
                in_=LATW[eb * EB:(eb + 1) * EB].rearrange("f s n -> s f n"),
            )
            for f in range(EB):
                e = eb * EB + f
                lhsT = xn_r[:, e, :]  # [S, B] strided
                for j, (n0, nn) in enumerate(NT):
                    nc.tensor.matmul(
                        lat_ps[j][:], lhsT, wt[:, f * NL + n0:f * NL + n0 + nn],
                        start=(e == 0), stop=False,
                    )
        for j, (n0, nn) in enumerate(NT):
            nc.tensor.matmul(
                lat_ps[j][:], ones_b[:], latb_sb[:, n0:n0 + nn],
                start=False, stop=True,
            )
        lat_part = sb.tile([B, NL], f32, tag="lat_part")
        for j, (n0, nn) in enumerate(NT):
            nc.vector.tensor_copy(lat_part[:, n0:n0 + nn], lat_ps[j][:])
        nc.sync.dma_start(out=a2a_in[:], in_=lat_part[:])
        nc.gpsimd.collective_compute(
            "AllToAll", mybir.AluOpType.bypass, replica_groups=groups,
            ins=[a2a_in.opt()], outs=[a2a_out.opt()],
        )

        scopeA.__exit__(None, None, None)
        scopeB = nc.named_scope("phaseB"); scopeB.__enter__()
        # =========== Phase B: attention for this core's batch ===========
        def load_w2(name, W, n, dt):
            w0 = wpool.tile([96, n], dt, tag=name + "0", name=name + "0")
            nc.scalar.dma_start(out=w0[:], in_=W[0:96, :])
            w1 = wpool.tile([97, n], dt, tag=name + "1", name=name + "1")
            nc.scalar.dma_start(out=w1[:], in_=W[96:193, :])
            return w0, w1

        qw0, qw1 = load_w2("qw", QW, E, bf16)
        kw0, kw1 = load_w2("kw", KW, E, bf16)
        vw0, vw1 = load_w2("vw", VW, E, bf16)
        ow0, ow1 = load_w2("ow", OW, E, bf16)
        cos_sb = wpool.tile([S, E // 2], f32, tag="cos_sb")
        nc.scalar.dma_start(out=cos_sb[:], in_=COS[:])
        sin_sb = wpool.tile([S, E // 2], f32, tag="sin_sb")
        nc.scalar.dma_start(out=sin_sb[:], in_=SIN[:])

        xb = sb.tile([S, E], f32, tag="xb")
        nc.scalar.dma_start(out=xb[:], in_=XB[:])
        # rmsnorm(xb)
        sqb = work.tile([S, E], f32, tag="sqb")
        msb = sb.tile([S, 1], f32, tag="msb")
        nc.scalar.activation(sqb[:], xb[:], AF.Square, accum_out=msb[:])
        rrb = sb.tile([S, 1], f32, tag="rrb")
        nc.scalar.activation(rrb[:], msb[:], AF.Sqrt, scale=1.0 / E, bias=eps_t[:])
        rib = sb.tile([S, 1], f32, tag="rib")
        nc.vector.reciprocal(rib[:], rrb[:])
        xnb = sb.tile([S, E], f32, tag="xnb")
        nc.vector.tensor_scalar_mul(xnb[:], xb[:], rib[:])

        # transpose xnb -> [e, s] bf16 K-tiles (96 rows + 97 rows with ones)
        xnbT0 = sb.tile([96, S], bf16, tag="xnbT0")
        xnbT1 = sb.tile([97, S], bf16, tag="xnbT1")
        pt0 = psP.tile([96, S], f32, tag="tp", bufs=2)
        nc.tensor.transpose(pt0[:], xnb[:, 0:96], idf[:])
        nc.vector.tensor_copy(xnbT0[:], pt0[:])
        pt1 = psP.tile([96, S], f32, tag="tp", bufs=2)
        nc.tensor.transpose(pt1[:], xnb[:, 96:192], idf[:])
        nc.vector.tensor_copy(xnbT1[0:96, :], pt1[:])
        nc.vector.memset(xnbT1[96:97, :], 1.0)

        # q = xnb @ qW + qb  -> [S, E]
        psq = psP.tile([S, E], f32, tag="mm", bufs=3)
        nc.tensor.matmul(psq[:], xnbT0[:], qw0[:], start=True, stop=False)
        nc.tensor.matmul(psq[:], xnbT1[:], qw1[:], start=False, stop=True)
        # RoPE directly from psum -> bf16 qr
        qr_bf = sb.tile([S, E], bf16, tag="qr_bf")
        q_pair = psq[:].rearrange("s (p two) -> s two p", two=2)
        qr_pair = qr_bf[:].rearrange("s (p two) -> s two p", two=2)
        t_a = work.tile([S, E // 2], f32, tag="t_a")
        t_b = work.tile([S, E // 2], f32, tag="t_b")
        nc.vector.tensor_mul(t_a[:], q_pair[:, 0, :], cos_sb[:])
        nc.vector.tensor_mul(t_b[:], q_pair[:, 1, :], sin_sb[:])
        nc.vector.tensor_sub(qr_pair[:, 0, :], t_a[:], t_b[:])
        t_c = work.tile([S, E // 2], f32, tag="t_c")
        t_d = work.tile([S, E // 2], f32, tag="t_d")
        nc.vector.tensor_mul(t_c[:], q_pair[:, 0, :], sin_sb[:])
        nc.vector.tensor_mul(t_d[:], q_pair[:, 1, :], cos_sb[:])
        nc.vector.tensor_add(qr_pair[:, 1, :], t_c[:], t_d[:])
        # transpose qr -> qrT [e, s] bf16
        qrT0 = sb.tile([96, S], bf16, tag="qrT0")
        qrT1 = sb.tile([96, S], bf16, tag="qrT1")
        ptq0 = psP.tile([96, S], bf16, tag="tp", bufs=2)
        nc.tensor.transpose(ptq0[:], qr_bf[:, 0:96], idb[:])
        nc.vector.tensor_copy(qrT0[:], ptq0[:])
        ptq1 = psP.tile([96, S], bf16, tag="tp", bufs=2)
        nc.tensor.transpose(ptq1[:], qr_bf[:, 96:192], idb[:])
        nc.vector.tensor_copy(qrT1[:], ptq1[:])

        # latent for this batch: [L, E] contiguous in a2a_out
        lat = sb.tile([L, E], f32, tag="lat")
        nc.sync.dma_start(out=lat[:], in_=a2a_out[:].rearrange("b n -> (b n)").rearrange("(l e) -> l e", e=E))
        latT0 = sb.tile([96, L], bf16, tag="latT0")
        latT1 = sb.tile([97, L], bf16, tag="latT1")
        ptl0 = psP.tile([96, L], f32, tag="tp", bufs=2)
        nc.tensor.transpose(ptl0[:], lat[:, 0:96], idf[0:64, 0:64])
        nc.vector.tensor_copy(latT0[:], ptl0[:])
        ptl1 = psP.tile([96, L], f32, tag="tp", bufs=2)
        nc.tensor.transpose(ptl1[:], lat[:, 96:192], idf[0:64, 0:64])
        nc.vector.tensor_copy(latT1[0:96, :], ptl1[:])
        nc.vector.memset(latT1[96:97, :], 1.0)

        # kT [e_out, l] bf16 (2 M-slices); v [l, e_out] bf16
        kT0 = sb.tile([96, L], bf16, tag="kT0")
        kT1 = sb.tile([96, L], bf16, tag="kT1")
        for mi, kT in enumerate((kT0, kT1)):
            pk = psP.tile([96, L], f32, tag="mm", bufs=3)
            nc.tensor.matmul(pk[:], kw0[:, mi * 96:(mi + 1) * 96], latT0[:],
                             start=True, stop=False)
            nc.tensor.matmul(pk[:], kw1[:, mi * 96:(mi + 1) * 96], latT1[:],
                             start=False, stop=True)
            nc.vector.tensor_copy(kT[:], pk[:])
        v_sb = sb.tile([L, E], bf16, tag="v_sb")
        pv = psP.tile([L, E], f32, tag="mm", bufs=3)
        nc.tensor.matmul(pv[:], latT0[:], vw0[:], start=True, stop=False)
        nc.tensor.matmul(pv[:], latT1[:], vw1[:], start=False, stop=True)
        nc.vector.tensor_copy(v_sb[:], pv[:])

        # attention heads
        aoT0 = sb.tile([96, S], bf16, tag="aoT0")
        aoT1 = sb.tile([97, S], bf16, tag="aoT1")
        nc.vector.memset(aoT1[96:97, :], 1.0)
        qrTs, kTs, aoTs = (qrT0, qrT1), (kT0, kT1), (aoT0, aoT1)
        for h in range(H):
            ht, hr = divmod(h, 3)
            pss = psP.tile([S, L], f32, tag="mm", bufs=3)
            nc.tensor.matmul(pss[:], qrTs[ht][hr * 32:(hr + 1) * 32, :],
                             kTs[ht][hr * 32:(hr + 1) * 32, :], start=True, stop=True)
            mx = work.tile([S, 1], f32, tag="mx")
            nc.vector.reduce_max(out=mx[:], in_=pss[:], axis=mybir.AxisListType.X)
            nbias = work.tile([S, 1], f32, tag="nbias")
            nc.vector.tensor_scalar_mul(nbias[:], mx[:], -ISCALE)
            p_sb = work.tile([S, L], f32, tag="p_sb")
            ssum = work.tile([S, 1], f32, tag="ssum")
            nc.scalar.activation(p_sb[:], pss[:], AF.Exp, scale=ISCALE,
                                 bias=nbias[:], accum_out=ssum[:])
            rsum = work.tile([S, 1], f32, tag="rsum")
            nc.vector.reciprocal(rsum[:], ssum[:])
            at_bf = work.tile([S, L], bf16, tag="at_bf")
            nc.vector.tensor_scalar_mul(at_bf[:], p_sb[:], rsum[:])
            psat = psP.tile([L, S], bf16, tag="tp", bufs=2)
            nc.tensor.transpose(psat[:], at_bf[:], idb[:])
            atT = work.tile([L, S], bf16, tag="atT")
            nc.vector.tensor_copy(atT[:], psat[:])
            pso = psP.tile([32, S], f32, tag="mm", bufs=3)
            nc.tensor.matmul(pso[:], v_sb[:, h * 32:(h + 1) * 32], atT[:],
                             start=True, stop=True)
            nc.vector.tensor_copy(aoTs[ht][hr * 32:(hr + 1) * 32, :], pso[:])

        # o proj + residual
        pso1 = psP.tile([S, E], f32, tag="mm", bufs=3)
        nc.tensor.matmul(pso1[:], aoT0[:], ow0[:], start=True, stop=False)
        nc.tensor.matmul(pso1[:], aoT1[:], ow1[:], start=False, stop=True)
        out1 = sb.tile([S, E], f32, tag="out1")
        nc.vector.tensor_add(out1[:], pso1[:], xb[:])
        nc.sync.dma_start(out=ag_in[:], in_=out1[:])
        nc.gpsimd.collective_compute(
            "AllGather", mybir.AluOpType.bypass, replica_groups=groups,
            ins=[ag_in.opt()], outs=[h_all.opt()],
        )

        scopeB.__exit__(None, None, None)
        scopeC = nc.named_scope("phaseC"); scopeC.__enter__()
        # =========== Phase C: MoE expert for this core, all tokens ===========
        rw0, rw1 = load_w2("rw", RW, NEXP, f32)
        e1w0, e1w1 = load_w2("e1w", E1W, ED, bf16)
        sww_k = []
        for k in range(6):
            t = wpool.tile([128, ED], bf16, tag=f"sww{k}", name=f"sww{k}")
            nc.scalar.dma_start(out=t[:], in_=SWW[k * 128:(k + 1) * 128, :])
            sww_k.append(t)
        swb_sb = wpool.tile([1, ED], bf16, tag="swb_sb")
        nc.scalar.dma_start(out=swb_sb[:], in_=SWW[ED:ED + 1, :])
        e2w_k = []
        for k in range(6):
            t = wpool.tile([128, E], bf16, tag=f"e2w{k}", name=f"e2w{k}")
            nc.scalar.dma_start(out=t[:], in_=E2W[k * 128:(k + 1) * 128, :])
            e2w_k.append(t)
        e2b_sb = wpool.tile([1, E], bf16, tag="e2b_sb")
        nc.scalar.dma_start(out=e2b_sb[:], in_=E2W[ED:ED + 1, :])

        S2 = 2 * S  # token-tile pair width
        for tp in range(B // 2):
            hin = work.tile([S, 2 * E], f32, tag="hin")  # two tiles side by side
            nc.sync.dma_start(
                out=hin[:].rearrange("s (u e) -> s u e", u=2),
                in_=h_all[2 * tp * S:(2 * tp + 2) * S, :].rearrange("(u s) e -> s u e", u=2),
            )
            msm = work.tile([S, 2], f32, tag="msm")
            for u in range(2):
                sqm = work.tile([S, E], f32, tag="sqm")
                nc.scalar.activation(sqm[:], hin[:, u * E:(u + 1) * E], AF.Square,
                                     accum_out=msm[:, u:u + 1])
            rrm = work.tile([S, 2], f32, tag="rrm")
            nc.scalar.activation(rrm[:], msm[:], AF.Sqrt, scale=1.0 / E, bias=eps_t[:])
            rim = work.tile([S, 2], f32, tag="rim")
            nc.vector.reciprocal(rim[:], rrm[:])
            hn = work.tile([S, 2 * E], f32, tag="hn")
            for u in range(2):
                nc.vector.tensor_scalar_mul(
                    hn[:, u * E:(u + 1) * E], hin[:, u * E:(u + 1) * E],
                    rim[:, u:u + 1])

            # transposes: [e, (u s)] bf16 + f32; rows 0:96 and 96:192(+ones)
            hnT0 = work.tile([96, S2], bf16, tag="hnT0")
            hnT1 = work.tile([97, S2], bf16, tag="hnT1")
            hnTf0 = work.tile([96, S2], f32, tag="hnTf0")
            hnTf1 = work.tile([97, S2], f32, tag="hnTf1")
            for u in range(2):
                ptm0 = psP.tile([96, S], f32, tag="tp", bufs=2)
                nc.tensor.transpose(ptm0[:], hn[:, u * E:u * E + 96], idf[:])
                nc.vector.tensor_copy(hnT0[:, u * S:(u + 1) * S], ptm0[:])
                nc.vector.tensor_copy(hnTf0[:, u * S:(u + 1) * S], ptm0[:])
                ptm1 = psP.tile([96, S], f32, tag="tp", bufs=2)
                nc.tensor.transpose(ptm1[:], hn[:, u * E + 96:u * E + 192], idf[:])
                nc.vector.tensor_copy(hnT1[0:96, u * S:(u + 1) * S], ptm1[:])
                nc.vector.tensor_copy(hnTf1[0:96, u * S:(u + 1) * S], ptm1[:])
            nc.vector.memset(hnT1[96:97, :], 1.0)
            nc.vector.memset(hnTf1[96:97, :], 1.0)

            # router (fp32): logits -> softmax -> top2 weight for expert 0 (rotated)
            plog = psP.tile([S, 2 * NEXP], f32, tag="mm", bufs=3)
            for u in range(2):
                nc.tensor.matmul(plog[:, u * NEXP:(u + 1) * NEXP],
                                 hnTf0[:, u * S:(u + 1) * S], rw0[:],
                                 start=True, stop=False)
                nc.tensor.matmul(plog[:, u * NEXP:(u + 1) * NEXP],
                                 hnTf1[:, u * S:(u + 1) * S], rw1[:],
                                 start=False, stop=True)
            w0 = work.tile([S, 2], f32, tag="w0")
            for u in range(2):
                pl = plog[:, u * NEXP:(u + 1) * NEXP]
                mx1 = work.tile([S, 1], f32, tag="mx1")
                nc.vector.reduce_max(out=mx1[:], in_=pl, axis=mybir.AxisListType.X)
                nb1 = work.tile([S, 1], f32, tag="nb1")
                nc.vector.tensor_scalar_mul(nb1[:], mx1[:], -1.0)
                pr = work.tile([S, NEXP], f32, tag="pr")
                sden = work.tile([S, 1], f32, tag="sden")
                nc.scalar.activation(pr[:], pl, AF.Exp, bias=nb1[:], accum_out=sden[:])
                rden = work.tile([S, 1], f32, tag="rden")
                nc.vector.reciprocal(rden[:], sden[:])
                probs = work.tile([S, NEXP], f32, tag="probs")
                nc.vector.tensor_scalar_mul(probs[:], pr[:], rden[:])
                pmax = work.tile([S, 1], f32, tag="pmax")
                nc.vector.reduce_max(out=pmax[:], in_=probs[:], axis=mybir.AxisListType.X)
                eqm = work.tile([S, NEXP], f32, tag="eqm")
                nc.vector.tensor_scalar(eqm[:], probs[:], pmax[:], None,
                                        op0=mybir.AluOpType.is_ge)
                knock = work.tile([S, NEXP], f32, tag="knock")
                nc.vector.tensor_scalar_mul(knock[:], eqm[:], 2.0)
                probs2 = work.tile([S, NEXP], f32, tag="probs2")
                nc.vector.tensor_sub(probs2[:], probs[:], knock[:])
                mx2 = work.tile([S, 1], f32, tag="mx2")
                nc.vector.reduce_max(out=mx2[:], in_=probs2[:], axis=mybir.AxisListType.X)
                m0 = work.tile([S, 1], f32, tag="m0")
                nc.vector.tensor_tensor(out=m0[:], in0=probs[:, 0:1], in1=mx2[:],
                                        op=mybir.AluOpType.is_ge)
                nc.vector.tensor_mul(w0[:, u:u + 1], probs[:, 0:1], m0[:])

            # expert: t1T slices (bf16), then t2T with relu, then eo token-major
            t1T = []
            for m in range(6):
                ps1 = psP.tile([128, S2], f32, tag="mm", bufs=3)
                nc.tensor.matmul(ps1[:], e1w0[:, m * 128:(m + 1) * 128], hnT0[:],
                                 start=True, stop=False)
                nc.tensor.matmul(ps1[:], e1w1[:, m * 128:(m + 1) * 128], hnT1[:],
                                 start=False, stop=True)
                t = work.tile([128, S2], bf16, tag=f"t1T{m}", name=f"t1T{m}")
                nc.vector.tensor_copy(t[:], ps1[:])
                t1T.append(t)
            t2T = []
            for m in range(6):
                ps2 = psP.tile([128, S2], f32, tag="mm", bufs=3)
                for k in range(6):
                    nc.tensor.matmul(ps2[:], sww_k[k][:, m * 128:(m + 1) * 128],
                                     t1T[k][:], start=(k == 0), stop=False)
                nc.tensor.matmul(ps2[:], swb_sb[:, m * 128:(m + 1) * 128], ones_s[:],
                                 start=False, stop=True)
                t = work.tile([128, S2], bf16, tag=f"t2T{m}", name=f"t2T{m}")
                nc.scalar.activation(t[:], ps2[:], AF.Relu)
                t2T.append(t)
            for u in range(2):
                pse = psP.tile([S, E], f32, tag="mm", bufs=3)
                for m in range(6):
                    nc.tensor.matmul(pse[:], t2T[m][:, u * S:(u + 1) * S], e2w_k[m][:],
                                     start=(m == 0), stop=False)
                nc.tensor.matmul(pse[:], ones_s[:, 0:S], e2b_sb[:],
                                 start=False, stop=True)
                y_t = work.tile([S, E], f32, tag="y_t")
                nc.vector.tensor_scalar_mul(y_t[:], pse[:], w0[:, u:u + 1])
                nc.sync.dma_start(out=rs_in[(2 * tp + u) * S:(2 * tp + u + 1) * S, :],
                                  in_=y_t[:])

        nc.gpsimd.collective_compute(
            "ReduceScatter", mybir.AluOpType.add, replica_groups=groups,
            ins=[rs_in.opt()], outs=[rs_out.opt()],
        )
        rsb = sb.tile([S, E], f32, tag="rsb")
        nc.sync.dma_start(out=rsb[:], in_=rs_out[:])
        fin = sb.tile([S, E], f32, tag="fin")
        nc.vector.tensor_add(fin[:], out1[:], rsb[:])
        nc.sync.dma_start(out=YB[:], in_=fin[:])
        scopeC.__exit__(None, None, None)

    nc.compile()
    return nc


def _prep_inputs(inputs):
    bf = ml_dtypes.bfloat16
    x = np.ascontiguousarray(inputs["x"], dtype=np.float32)
    rms1 = np.asarray(inputs["rms1_w"], dtype=np.float32)
    rms2 = np.asarray(inputs["rms2_w"], dtype=np.float32)
    latW = np.asarray(inputs["latW"], dtype=np.float32)
    latb = np.asarray(inputs["latb"], dtype=np.float32)
    qW, qb = np.asarray(inputs["qW"], np.float32), np.asarray(inputs["qb"], np.float32)
    kW, kb = np.asarray(inputs["kW"], np.float32), np.asarray(inputs["kb"], np.float32)
    vW, vb = np.asarray(inputs["vW"], np.float32), np.asarray(inputs["vb"], np.float32)
    oW, ob = np.asarray(inputs["oW"], np.float32), np.asarray(inputs["ob"], np.float32)
    rW, rb = np.asarray(inputs["rW"], np.float32), np.asarray(inputs["rb"], np.float32)
    e1W, e1b = np.asarray(inputs["e1W"], np.float32), np.asarray(inputs["e1b"], np.float32)
    swW, swb = np.asarray(inputs["swW"], np.float32), np.asarray(inputs["swb"], np.float32)
    e2W, e2b = np.asarray(inputs["e2W"], np.float32), np.asarray(inputs["e2b"], np.float32)

    # latW rows are (s, e)-major; fold rms1[e], permute to (e, s)-major, shard cols
    latWp = (latW.reshape(S, E, L * E) * rms1[None, :, None]).transpose(1, 0, 2)
    latW_bf = latWp.astype(bf)  # [E, S, L*E]

    qWp = np.concatenate([qW * rms1[:, None], qb[None]], 0).astype(bf)
    kWp = np.concatenate([kW, kb[None]], 0).astype(bf)
    vWp = np.concatenate([vW, vb[None]], 0).astype(bf)
    oWp = np.concatenate([oW, ob[None]], 0).astype(bf)
    rW_eff = np.concatenate([rW * rms2[:, None], rb[None]], 0).astype(np.float32)

    t = np.arange(S, dtype=np.float64)
    inv_freq = 1.0 / (10000.0 ** (np.arange(0, HD, 2, dtype=np.float64) / HD))
    freqs = np.outer(t, inv_freq)  # [S, 16]
    cos_full = np.tile(np.cos(freqs), (1, H)).astype(np.float32)
    sin_full = np.tile(np.sin(freqs), (1, H)).astype(np.float32)

    in_maps = []
    for c in range(NC):
        cols = np.roll(np.arange(NEXP), -c)
        m = {
            "X": x,
            "XB": np.ascontiguousarray(x[c]),
            "LATW": np.ascontiguousarray(latW_bf[:, :, c * NL:(c + 1) * NL]),
            "LATB": latb[c * NL:(c + 1) * NL].astype(bf)[None, :],
            "QW": qWp, "KW": kWp, "VW": vWp, "OW": oWp,
            "COS": cos_full, "SIN": sin_full,
            "RW": np.ascontiguousarray(rW_eff[:, cols]),
            "E1W": np.concatenate([e1W[c] * rms2[:, None], e1b[c][None]], 0).astype(bf),
            "SWW": np.concatenate([swW[c], swb[c][None]], 0).astype(bf),
            "E2W": np.concatenate([e2W[c], e2b[c][None]], 0).astype(bf),
        }
        in_maps.append(m)
    return in_maps


def _get_program():
    if "nc" not in _cache:
        _cache["nc"] = _build_program()
    return _cache["nc"]


def run(inputs, trace=False):
    nc = _get_program()
    in_maps = _prep_inputs(inputs)
    res = run_bass_kernel_spmd(nc, in_maps, list(range(NC)), trace=trace)
    out = np.stack([res.results[c]["YB"] for c in range(NC)], axis=0)
    return out.astype(np.float32), res


def kernel(**inputs):
    out, _ = run(inputs, trace=False)
    return out
